# revision 26
# baseline (speedup 1.0000x reference)
import numpy as np

B, S, E, D = 32, 2048, 1024, 1024
N_CORES = 8
BL = B // N_CORES
SC = 512
N_SC = S // SC
EC = E // 128
DC = D // 128
ST = S // 128

_CACHE = {}


def _alu(name):
    import concourse.mybir as mybir
    return getattr(mybir.AluOpType, name)


def _axis(name):
    import concourse.mybir as mybir
    return getattr(mybir.AxisListType, name)


def _build():
    import concourse.bacc as bacc
    import concourse.mybir as mybir
    import concourse.tile as tile
    from concourse.masks import make_identity

    f32 = mybir.dt.float32
    f32r = mybir.dt.float32r
    bf16 = mybir.dt.bfloat16
    i32 = mybir.dt.int32
    Tanh = mybir.ActivationFunctionType.Tanh
    Exp = mybir.ActivationFunctionType.Exp
    Copy = mybir.ActivationFunctionType.Copy

    nc = bacc.Bacc("TRN2", target_bir_lowering=False, debug=False,
                   num_devices=N_CORES)

    decT_p = nc.dram_tensor("decT_p", [128, DC * BL], bf16,
                            kind="ExternalInput").ap()
    enc = nc.dram_tensor("enc", [BL, S, E], bf16, kind="ExternalInput").ap()
    mask = nc.dram_tensor("mask", [BL, S], i32, kind="ExternalInput").ap()
    w_encTd = nc.dram_tensor("w_encTd", [DC, E, 128], bf16,
                             kind="ExternalInput").ap()
    w_decT = nc.dram_tensor("w_decT", [D, D], bf16, kind="ExternalInput").ap()
    v_p = nc.dram_tensor("v_p", [128, DC], bf16, kind="ExternalInput").ap()
    ctx_out = nc.dram_tensor("context", [BL, E], f32r, kind="ExternalOutput").ap()
    wts_out = nc.dram_tensor("weights", [BL, S], f32r, kind="ExternalOutput").ap()

    with tile.TileContext(nc) as tc:
        with (
            tc.tile_pool(name="consts", bufs=1) as consts,
            tc.tile_pool(name="encres", bufs=2) as encres_pool,
            tc.tile_pool(name="enct", bufs=12) as enct_pool,
            tc.tile_pool(name="energy", bufs=10) as en_pool,
            tc.tile_pool(name="small", bufs=2) as small,
            tc.tile_pool(name="rows", bufs=4) as rows,
            tc.tile_pool(name="ps_tp", bufs=3, space="PSUM") as ps_tp,
            tc.tile_pool(name="ps_mm", bufs=2, space="PSUM") as ps_mm,
            tc.tile_pool(name="ps_vec", bufs=1, space="PSUM") as ps_vec,
            tc.tile_pool(name="ps_ctx", bufs=1, space="PSUM") as ps_ctx,
        ):
            ident_f = consts.tile([128, 128], f32)
            make_identity(nc, ident_f)
            ident_bf = consts.tile([128, 128], bf16)
            nc.vector.tensor_copy(ident_bf[:], ident_f[:])

            wencT = consts.tile([128, DC * EC * 128], bf16)
            wencT4 = wencT[:].rearrange("p (dj ei c) -> p dj ei c", dj=DC, ei=EC)
            enc_res0 = encres_pool.tile([128, ST * 1024], bf16, tag="encres")
            v_bf = consts.tile([128, DC], bf16)

            def load_enc0_pair(k):
                src0 = enc[0, k * 512:(k + 1) * 512, :]
                dst0 = enc_res0[:, k * 4096:(k + 1) * 4096]
                nc.sync.dma_start(
                    out=dst0.rearrange("p (t e) -> p t e", t=4),
                    in_=src0.rearrange("(t p) e -> p t e", p=128))

            def load_wenc_pair(k):
                nc.sync.dma_start(
                    out=wencT4[:, 2 * k:2 * k + 2, :, :],
                    in_=w_encTd[2 * k:2 * k + 2]
                        .rearrange("j (ei p) c -> p j ei c", p=128))

            load_enc0_pair(0)
            load_wenc_pair(0)
            dhT = consts.tile([128, BL * DC], f32)
            wdec_stage = encres_pool.tile([128, DC * 1024], bf16, tag="encres")
            wdec3 = wdec_stage[:].rearrange("p (jc d) -> p jc d", jc=DC)
            nc.sync.dma_start(
                out=wdec3,
                in_=w_decT.rearrange("(jc p) d -> p jc d", p=128))
            decT = consts.tile([128, BL * DC], bf16)
            nc.sync.dma_start(out=decT[:], in_=decT_p[:, :])
            nc.sync.dma_start(out=v_bf[:], in_=v_p[:, :])
            for k in range(1, 4):
                load_wenc_pair(k)
                load_enc0_pair(k)

            def emit_dh():
                ps_db = ps_ctx.tile([BL, 1024], f32, tag="ctx")
                for jc in range(DC):
                    for h in range(2):
                        nc.tensor.matmul(ps_db[:, h * 512:(h + 1) * 512],
                                         decT[:, jc * BL:(jc + 1) * BL],
                                         wdec3[:, jc, h * 512:(h + 1) * 512],
                                         start=(jc == 0), stop=(jc == DC - 1))
                dh_bd = small.tile([BL, 1024], f32, tag="dhbd", bufs=1)
                nc.scalar.copy(dh_bd[:], ps_db[:])
                ps_dt = ps_tp.tile([128, BL * DC], f32, tag="tp")
                for dj in range(DC):
                    nc.tensor.transpose(ps_dt[:, dj * BL:(dj + 1) * BL],
                                        dh_bd[:, dj * 128:(dj + 1) * 128],
                                        ident_f[:BL, :BL])
                nc.scalar.copy(dhT[:], ps_dt[:])

            def phase_E(b, c_callback=None, post_tp0=None, preloaded=None):
                if preloaded is not None:
                    enc_res = preloaded
                else:
                    enc_res = encres_pool.tile([128, ST * 1024], bf16,
                                               tag="encres")
                    for ld in range(4):
                        src = enc[b, ld * 512:(ld + 1) * 512, :]
                        dst = enc_res[:, ld * 4096:(ld + 1) * 4096]
                        nc.sync.dma_start(
                            out=dst.rearrange("p (t e) -> p t e", t=4),
                            in_=src.rearrange("(t p) e -> p t e", p=128))
                mask_ib = small.tile([1, S], i32, tag="mask")
                nc.sync.dma_start(out=mask_ib[:], in_=mask[b:b + 1, :])
                sm = rows.tile([1, S], f32, tag="row")
                wexp = rows.tile([1, S], f32, tag="row")
                se4 = small.tile([1, N_SC], f32, tag="pmax")
                for sc in range(N_SC):
                    if sc == 1 and c_callback is not None:
                        c_callback()
                    enct = []
                    for ep in range(EC // 2):
                        ps = ps_tp.tile([128, 1024], bf16, tag="tp")
                        for half in range(2):
                            ei = 2 * ep + half
                            for q in range(4):
                                t_abs = sc * 4 + q
                                src = enc_res[:, t_abs * 1024 + ei * 128:
                                              t_abs * 1024 + (ei + 1) * 128]
                                nc.tensor.transpose(
                                    ps[:, half * 512 + q * 128:
                                       half * 512 + (q + 1) * 128],
                                    src, ident_bf)
                        for half in range(2):
                            et = enct_pool.tile([128, 512], bf16, tag="enct")
                            nc.vector.tensor_copy(
                                et[:], ps[:, half * 512:(half + 1) * 512])
                            enct.append(et)
                    if sc == 0 and post_tp0 is not None:
                        post_tp0()
                    ens = []
                    for dj in range(DC):
                        psm = ps_mm.tile([128, 512], f32, tag="mm")
                        for ei in range(EC):
                            nc.tensor.matmul(
                                psm[:], wencT4[:, dj, ei, :], enct[ei][:],
                                start=(ei == 0), stop=(ei == EC - 1))
                        en = en_pool.tile([128, 512], bf16, tag="energy")
                        nc.scalar.activation(en[:], psm[:], Tanh,
                                             bias=dhT[:, dj * BL + b: dj * BL + b + 1])
                        ens.append(en)
                    ps_s = ps_vec.tile([1, 512], f32, tag="vec")
                    for dj in range(DC):
                        nc.tensor.matmul(ps_s[:], v_bf[:, dj:dj + 1],
                                         ens[dj][:],
                                         start=(dj == 0), stop=(dj == DC - 1))
                    pen = small.tile([1, 512], f32, tag="pen")
                    nc.vector.tensor_scalar(
                        out=pen[:], in0=mask_ib[:, sc * SC:(sc + 1) * SC],
                        scalar1=1e9, scalar2=-(1e9 + 8.0),
                        op0=_alu("mult"), op1=_alu("add"))
                    nc.vector.tensor_tensor(
                        out=sm[:, sc * SC:(sc + 1) * SC], in0=ps_s[:], in1=pen[:],
                        op=_alu("add"))
                    nc.scalar.activation(wexp[:, sc * SC:(sc + 1) * SC],
                                         sm[:, sc * SC:(sc + 1) * SC], Exp,
                                         accum_out=se4[:, sc:sc + 1])
                sumexp = small.tile([1, 1], f32, tag="sumexp")
                nc.vector.reduce_sum(sumexp[:], se4[:], axis=_axis("X"))
                rec = small.tile([1, 1], f32, tag="rec")
                nc.vector.reciprocal(rec[:], sumexp[:])
                wnorm = rows.tile([1, S], f32, tag="row")
                nc.vector.tensor_scalar_mul(wnorm[:], wexp[:], rec[:])
                nc.sync.dma_start(out=wts_out[b:b + 1, :],
                                  in_=wnorm[:].bitcast(f32r))
                return {"wexp": wexp, "rec": rec, "enc_res": enc_res}

            def phase_C(b, st_b):
                wexp, rec, enc_res = st_b["wexp"], st_b["rec"], st_b["enc_res"]
                ps_w = ps_tp.tile([128, ST], f32, tag="tp")
                for t in range(ST):
                    nc.tensor.transpose(ps_w[:, t:t + 1],
                                        wexp[:, t * 128:(t + 1) * 128],
                                        ident_f[:1, :1])
                wT = small.tile([128, ST], bf16, tag="wt")
                nc.vector.tensor_copy(wT[:], ps_w[:])

                psc = ps_ctx.tile([1, 1024], f32, tag="ctx")
                for st in range(ST):
                    rhs = enc_res[:, st * 1024:(st + 1) * 1024]
                    for h in range(2):
                        nc.tensor.matmul(psc[:, h * 512:(h + 1) * 512],
                                         wT[:, st:st + 1],
                                         rhs[:, h * 512:(h + 1) * 512],
                                         start=(st == 0), stop=(st == ST - 1))
                ctx_sb = small.tile([1, 1024], f32r, tag="ctxsb")
                nc.scalar.activation(ctx_sb[:, 0:512], psc[:, 0:512], Copy,
                                     scale=rec[:])
                nc.scalar.activation(ctx_sb[:, 512:1024], psc[:, 512:1024], Copy,
                                     scale=rec[:])
                nc.sync.dma_start(out=ctx_out[b:b + 1, :], in_=ctx_sb[:])

            states = {}
            for b in range(BL):
                cb = None
                if b >= 1:
                    cb = (lambda bb=b: phase_C(bb - 1, states[bb - 1]))
                states[b] = phase_E(b, c_callback=cb,
                                    post_tp0=emit_dh if b == 0 else None,
                                    preloaded=enc_res0 if b == 0 else None)
            phase_C(BL - 1, states[BL - 1])

    nc.compile()
    from concourse.bass_interp import get_hw_module
    nc.m = get_hw_module(nc.m)
    return nc


def get_nc():
    if "nc" not in _CACHE:
        _CACHE["nc"] = _build()
    return _CACHE["nc"]


def make_in_maps(dec_hidden, enc_outputs, mask, W_enc, W_dec, v):
    dec_hidden = np.ascontiguousarray(dec_hidden, dtype=np.float32)
    enc_outputs = np.ascontiguousarray(enc_outputs, dtype=np.float32)
    mask = np.ascontiguousarray(mask, dtype=np.int32)
    import ml_dtypes
    bf = ml_dtypes.bfloat16
    W_enc = np.asarray(W_enc, dtype=np.float32)
    W_dec = np.asarray(W_dec, dtype=np.float32)
    enc_bf = enc_outputs.astype(bf)
    w_encTd = np.ascontiguousarray(
        W_enc.T.reshape(E, DC, 128).transpose(1, 0, 2)).astype(bf)
    w_decT = np.ascontiguousarray(W_dec.T).astype(bf)
    v_p = np.ascontiguousarray(
        np.asarray(v, dtype=np.float32).reshape(DC, 128).T).astype(bf)

    in_maps = []
    for c in range(N_CORES):
        sl = slice(c * BL, (c + 1) * BL)
        decT_p = np.ascontiguousarray(
            dec_hidden[sl].T.reshape(DC, 128, BL).transpose(1, 0, 2)
            .reshape(128, DC * BL)).astype(bf)
        in_maps.append({
            "decT_p": decT_p,
            "enc": np.ascontiguousarray(enc_bf[sl]),
            "mask": np.ascontiguousarray(mask[sl]),
            "w_encTd": w_encTd, "w_decT": w_decT, "v_p": v_p,
        })
    return in_maps


def kernel(dec_hidden, enc_outputs, mask, W_enc, W_dec, v):
    from concourse import bass_utils

    nc = get_nc()
    in_maps = make_in_maps(dec_hidden, enc_outputs, mask, W_enc, W_dec, v)
    res = bass_utils.run_bass_kernel_spmd(nc, in_maps,
                                          core_ids=list(range(N_CORES)))
    context = np.concatenate([res.results[c]["context"] for c in range(N_CORES)])
    weights = np.concatenate([res.results[c]["weights"] for c in range(N_CORES)])
    return context.astype(np.float32), weights.astype(np.float32)


# revision 27
# speedup vs baseline: 1.0088x; 1.0088x over previous
import numpy as np

B, S, E, D = 32, 2048, 1024, 1024
N_CORES = 8
BL = B // N_CORES
SC = 512
N_SC = S // SC
EC = E // 128
DC = D // 128
ST = S // 128

_CACHE = {}


def _alu(name):
    import concourse.mybir as mybir
    return getattr(mybir.AluOpType, name)


def _axis(name):
    import concourse.mybir as mybir
    return getattr(mybir.AxisListType, name)


def _build():
    import concourse.bacc as bacc
    import concourse.mybir as mybir
    import concourse.tile as tile
    from concourse.masks import make_identity

    f32 = mybir.dt.float32
    f32r = mybir.dt.float32r
    bf16 = mybir.dt.bfloat16
    i32 = mybir.dt.int32
    Tanh = mybir.ActivationFunctionType.Tanh
    Exp = mybir.ActivationFunctionType.Exp
    Copy = mybir.ActivationFunctionType.Copy

    nc = bacc.Bacc("TRN2", target_bir_lowering=False, debug=False,
                   num_devices=N_CORES)

    decT_p = nc.dram_tensor("decT_p", [128, DC * BL], bf16,
                            kind="ExternalInput").ap()
    enc = nc.dram_tensor("enc", [BL, S, E], bf16, kind="ExternalInput").ap()
    mask = nc.dram_tensor("mask", [BL, S], i32, kind="ExternalInput").ap()
    w_encTd = nc.dram_tensor("w_encTd", [DC, E, 128], bf16,
                             kind="ExternalInput").ap()
    w_decT = nc.dram_tensor("w_decT", [D, D], bf16, kind="ExternalInput").ap()
    v_p = nc.dram_tensor("v_p", [128, DC], bf16, kind="ExternalInput").ap()
    ctx_out = nc.dram_tensor("context", [BL, E], f32r, kind="ExternalOutput").ap()
    wts_out = nc.dram_tensor("weights", [BL, S], f32r, kind="ExternalOutput").ap()

    with tile.TileContext(nc) as tc:
        with (
            tc.tile_pool(name="consts", bufs=1) as consts,
            tc.tile_pool(name="encres", bufs=2) as encres_pool,
            tc.tile_pool(name="enct", bufs=12) as enct_pool,
            tc.tile_pool(name="energy", bufs=10) as en_pool,
            tc.tile_pool(name="small", bufs=2) as small,
            tc.tile_pool(name="rows", bufs=4) as rows,
            tc.tile_pool(name="ps_tp", bufs=3, space="PSUM") as ps_tp,
            tc.tile_pool(name="ps_mm", bufs=2, space="PSUM") as ps_mm,
            tc.tile_pool(name="ps_vec", bufs=1, space="PSUM") as ps_vec,
            tc.tile_pool(name="ps_ctx", bufs=1, space="PSUM") as ps_ctx,
        ):
            ident_f = consts.tile([128, 128], f32)
            make_identity(nc, ident_f)
            ident_bf = consts.tile([128, 128], bf16)
            nc.vector.tensor_copy(ident_bf[:], ident_f[:])

            wencT = consts.tile([128, DC * EC * 128], bf16)
            wencT4 = wencT[:].rearrange("p (dj ei c) -> p dj ei c", dj=DC, ei=EC)
            enc_res0 = encres_pool.tile([128, ST * 1024], bf16, tag="encres")
            v_bf = consts.tile([128, DC], bf16)

            def load_enc0_pair(k):
                src0 = enc[0, k * 512:(k + 1) * 512, :]
                dst0 = enc_res0[:, k * 4096:(k + 1) * 4096]
                nc.sync.dma_start(
                    out=dst0.rearrange("p (t e) -> p t e", t=4),
                    in_=src0.rearrange("(t p) e -> p t e", p=128))

            def load_wenc_pair(k):
                nc.sync.dma_start(
                    out=wencT4[:, 2 * k:2 * k + 2, :, :],
                    in_=w_encTd[2 * k:2 * k + 2]
                        .rearrange("j (ei p) c -> p j ei c", p=128))

            load_enc0_pair(0)
            dhT = consts.tile([128, BL * DC], f32)
            wdec_stage = encres_pool.tile([128, DC * 1024], bf16, tag="encres")
            wdec3 = wdec_stage[:].rearrange("p (jc d) -> p jc d", jc=DC)
            decT = consts.tile([128, BL * DC], bf16)
            nc.sync.dma_start(out=decT[:], in_=decT_p[:, :])
            nc.sync.dma_start(
                out=wdec3[:, 0:DC // 2, :],
                in_=w_decT[0:D // 2].rearrange("(jc p) d -> p jc d", p=128))
            load_wenc_pair(0)
            nc.sync.dma_start(
                out=wdec3[:, DC // 2:DC, :],
                in_=w_decT[D // 2:D].rearrange("(jc p) d -> p jc d", p=128))
            nc.sync.dma_start(out=v_bf[:], in_=v_p[:, :])
            for k in range(1, 4):
                load_wenc_pair(k)
                load_enc0_pair(k)

            def emit_dh():
                ps_db = ps_ctx.tile([BL, 1024], f32, tag="ctx")
                for jc in range(DC):
                    for h in range(2):
                        nc.tensor.matmul(ps_db[:, h * 512:(h + 1) * 512],
                                         decT[:, jc * BL:(jc + 1) * BL],
                                         wdec3[:, jc, h * 512:(h + 1) * 512],
                                         start=(jc == 0), stop=(jc == DC - 1))
                dh_bd = small.tile([BL, 1024], f32, tag="dhbd", bufs=1)
                nc.scalar.copy(dh_bd[:], ps_db[:])
                ps_dt = ps_tp.tile([128, BL * DC], f32, tag="tp")
                for dj in range(DC):
                    nc.tensor.transpose(ps_dt[:, dj * BL:(dj + 1) * BL],
                                        dh_bd[:, dj * 128:(dj + 1) * 128],
                                        ident_f[:BL, :BL])
                nc.scalar.copy(dhT[:], ps_dt[:])

            def phase_E(b, c_callback=None, post_tp0=None, preloaded=None):
                if preloaded is not None:
                    enc_res = preloaded
                else:
                    enc_res = encres_pool.tile([128, ST * 1024], bf16,
                                               tag="encres")
                    for ld in range(4):
                        src = enc[b, ld * 512:(ld + 1) * 512, :]
                        dst = enc_res[:, ld * 4096:(ld + 1) * 4096]
                        nc.sync.dma_start(
                            out=dst.rearrange("p (t e) -> p t e", t=4),
                            in_=src.rearrange("(t p) e -> p t e", p=128))
                mask_ib = small.tile([1, S], i32, tag="mask")
                nc.sync.dma_start(out=mask_ib[:], in_=mask[b:b + 1, :])
                sm = rows.tile([1, S], f32, tag="row")
                wexp = rows.tile([1, S], f32, tag="row")
                se4 = small.tile([1, N_SC], f32, tag="pmax")
                for sc in range(N_SC):
                    if sc == 1 and c_callback is not None:
                        c_callback()
                    enct = []
                    for ep in range(EC // 2):
                        ps = ps_tp.tile([128, 1024], bf16, tag="tp")
                        for half in range(2):
                            ei = 2 * ep + half
                            for q in range(4):
                                t_abs = sc * 4 + q
                                src = enc_res[:, t_abs * 1024 + ei * 128:
                                              t_abs * 1024 + (ei + 1) * 128]
                                nc.tensor.transpose(
                                    ps[:, half * 512 + q * 128:
                                       half * 512 + (q + 1) * 128],
                                    src, ident_bf)
                        for half in range(2):
                            et = enct_pool.tile([128, 512], bf16, tag="enct")
                            nc.vector.tensor_copy(
                                et[:], ps[:, half * 512:(half + 1) * 512])
                            enct.append(et)
                    if sc == 0 and post_tp0 is not None:
                        post_tp0()
                    ens = []
                    for dj in range(DC):
                        psm = ps_mm.tile([128, 512], f32, tag="mm")
                        for ei in range(EC):
                            nc.tensor.matmul(
                                psm[:], wencT4[:, dj, ei, :], enct[ei][:],
                                start=(ei == 0), stop=(ei == EC - 1))
                        en = en_pool.tile([128, 512], bf16, tag="energy")
                        nc.scalar.activation(en[:], psm[:], Tanh,
                                             bias=dhT[:, dj * BL + b: dj * BL + b + 1])
                        ens.append(en)
                    ps_s = ps_vec.tile([1, 512], f32, tag="vec")
                    for dj in range(DC):
                        nc.tensor.matmul(ps_s[:], v_bf[:, dj:dj + 1],
                                         ens[dj][:],
                                         start=(dj == 0), stop=(dj == DC - 1))
                    pen = small.tile([1, 512], f32, tag="pen")
                    nc.vector.tensor_scalar(
                        out=pen[:], in0=mask_ib[:, sc * SC:(sc + 1) * SC],
                        scalar1=1e9, scalar2=-(1e9 + 8.0),
                        op0=_alu("mult"), op1=_alu("add"))
                    nc.vector.tensor_tensor(
                        out=sm[:, sc * SC:(sc + 1) * SC], in0=ps_s[:], in1=pen[:],
                        op=_alu("add"))
                    nc.scalar.activation(wexp[:, sc * SC:(sc + 1) * SC],
                                         sm[:, sc * SC:(sc + 1) * SC], Exp,
                                         accum_out=se4[:, sc:sc + 1])
                sumexp = small.tile([1, 1], f32, tag="sumexp")
                nc.vector.reduce_sum(sumexp[:], se4[:], axis=_axis("X"))
                rec = small.tile([1, 1], f32, tag="rec")
                nc.vector.reciprocal(rec[:], sumexp[:])
                wnorm = rows.tile([1, S], f32, tag="row")
                nc.vector.tensor_scalar_mul(wnorm[:], wexp[:], rec[:])
                nc.sync.dma_start(out=wts_out[b:b + 1, :],
                                  in_=wnorm[:].bitcast(f32r))
                return {"wexp": wexp, "rec": rec, "enc_res": enc_res}

            def phase_C(b, st_b):
                wexp, rec, enc_res = st_b["wexp"], st_b["rec"], st_b["enc_res"]
                ps_w = ps_tp.tile([128, ST], f32, tag="tp")
                for t in range(ST):
                    nc.tensor.transpose(ps_w[:, t:t + 1],
                                        wexp[:, t * 128:(t + 1) * 128],
                                        ident_f[:1, :1])
                wT = small.tile([128, ST], bf16, tag="wt")
                nc.vector.tensor_copy(wT[:], ps_w[:])

                psc = ps_ctx.tile([1, 1024], f32, tag="ctx")
                for st in range(ST):
                    rhs = enc_res[:, st * 1024:(st + 1) * 1024]
                    for h in range(2):
                        nc.tensor.matmul(psc[:, h * 512:(h + 1) * 512],
                                         wT[:, st:st + 1],
                                         rhs[:, h * 512:(h + 1) * 512],
                                         start=(st == 0), stop=(st == ST - 1))
                ctx_sb = small.tile([1, 1024], f32r, tag="ctxsb")
                nc.scalar.activation(ctx_sb[:, 0:512], psc[:, 0:512], Copy,
                                     scale=rec[:])
                nc.scalar.activation(ctx_sb[:, 512:1024], psc[:, 512:1024], Copy,
                                     scale=rec[:])
                nc.sync.dma_start(out=ctx_out[b:b + 1, :], in_=ctx_sb[:])

            states = {}
            for b in range(BL):
                cb = None
                if b >= 1:
                    cb = (lambda bb=b: phase_C(bb - 1, states[bb - 1]))
                states[b] = phase_E(b, c_callback=cb,
                                    post_tp0=emit_dh if b == 0 else None,
                                    preloaded=enc_res0 if b == 0 else None)
            phase_C(BL - 1, states[BL - 1])

    nc.compile()
    from concourse.bass_interp import get_hw_module
    nc.m = get_hw_module(nc.m)
    return nc


def get_nc():
    if "nc" not in _CACHE:
        _CACHE["nc"] = _build()
    return _CACHE["nc"]


def make_in_maps(dec_hidden, enc_outputs, mask, W_enc, W_dec, v):
    dec_hidden = np.ascontiguousarray(dec_hidden, dtype=np.float32)
    enc_outputs = np.ascontiguousarray(enc_outputs, dtype=np.float32)
    mask = np.ascontiguousarray(mask, dtype=np.int32)
    import ml_dtypes
    bf = ml_dtypes.bfloat16
    W_enc = np.asarray(W_enc, dtype=np.float32)
    W_dec = np.asarray(W_dec, dtype=np.float32)
    enc_bf = enc_outputs.astype(bf)
    w_encTd = np.ascontiguousarray(
        W_enc.T.reshape(E, DC, 128).transpose(1, 0, 2)).astype(bf)
    w_decT = np.ascontiguousarray(W_dec.T).astype(bf)
    v_p = np.ascontiguousarray(
        np.asarray(v, dtype=np.float32).reshape(DC, 128).T).astype(bf)

    in_maps = []
    for c in range(N_CORES):
        sl = slice(c * BL, (c + 1) * BL)
        decT_p = np.ascontiguousarray(
            dec_hidden[sl].T.reshape(DC, 128, BL).transpose(1, 0, 2)
            .reshape(128, DC * BL)).astype(bf)
        in_maps.append({
            "decT_p": decT_p,
            "enc": np.ascontiguousarray(enc_bf[sl]),
            "mask": np.ascontiguousarray(mask[sl]),
            "w_encTd": w_encTd, "w_decT": w_decT, "v_p": v_p,
        })
    return in_maps


def kernel(dec_hidden, enc_outputs, mask, W_enc, W_dec, v):
    from concourse import bass_utils

    nc = get_nc()
    in_maps = make_in_maps(dec_hidden, enc_outputs, mask, W_enc, W_dec, v)
    res = bass_utils.run_bass_kernel_spmd(nc, in_maps,
                                          core_ids=list(range(N_CORES)))
    context = np.concatenate([res.results[c]["context"] for c in range(N_CORES)])
    weights = np.concatenate([res.results[c]["weights"] for c in range(N_CORES)])
    return context.astype(np.float32), weights.astype(np.float32)


# revision 31
# speedup vs baseline: 1.0168x; 1.0080x over previous
import numpy as np

B, S, E, D = 32, 2048, 1024, 1024
N_CORES = 8
BL = B // N_CORES
SC = 512
N_SC = S // SC
EC = E // 128
DC = D // 128
ST = S // 128

_CACHE = {}


def _alu(name):
    import concourse.mybir as mybir
    return getattr(mybir.AluOpType, name)


def _axis(name):
    import concourse.mybir as mybir
    return getattr(mybir.AxisListType, name)


def _build():
    import concourse.bacc as bacc
    import concourse.mybir as mybir
    import concourse.tile as tile
    from concourse.masks import make_identity

    f32 = mybir.dt.float32
    f32r = mybir.dt.float32r
    bf16 = mybir.dt.bfloat16
    i32 = mybir.dt.int32
    Tanh = mybir.ActivationFunctionType.Tanh
    Exp = mybir.ActivationFunctionType.Exp
    Copy = mybir.ActivationFunctionType.Copy

    nc = bacc.Bacc("TRN2", target_bir_lowering=False, debug=False,
                   num_devices=N_CORES)

    decT_p = nc.dram_tensor("decT_p", [128, DC * BL], bf16,
                            kind="ExternalInput").ap()
    enc = nc.dram_tensor("enc", [BL, S, E], bf16, kind="ExternalInput").ap()
    mask = nc.dram_tensor("mask", [BL, S], i32, kind="ExternalInput").ap()
    w_encTd = nc.dram_tensor("w_encTd", [DC, E, 128], bf16,
                             kind="ExternalInput").ap()
    w_decT = nc.dram_tensor("w_decT", [D, D], bf16, kind="ExternalInput").ap()
    v_p = nc.dram_tensor("v_p", [128, DC], bf16, kind="ExternalInput").ap()
    ctx_out = nc.dram_tensor("context", [BL, E], f32r, kind="ExternalOutput").ap()
    wts_out = nc.dram_tensor("weights", [BL, S], f32r, kind="ExternalOutput").ap()

    with tile.TileContext(nc) as tc:
        with (
            tc.tile_pool(name="consts", bufs=1) as consts,
            tc.tile_pool(name="encres", bufs=2) as encres_pool,
            tc.tile_pool(name="enct", bufs=12) as enct_pool,
            tc.tile_pool(name="energy", bufs=10) as en_pool,
            tc.tile_pool(name="small", bufs=2) as small,
            tc.tile_pool(name="rows", bufs=4) as rows,
            tc.tile_pool(name="ps_tp", bufs=3, space="PSUM") as ps_tp,
            tc.tile_pool(name="ps_mm", bufs=2, space="PSUM") as ps_mm,
            tc.tile_pool(name="ps_vec", bufs=1, space="PSUM") as ps_vec,
            tc.tile_pool(name="ps_ctx", bufs=1, space="PSUM") as ps_ctx,
        ):
            ident_f = consts.tile([128, 128], f32)
            make_identity(nc, ident_f)
            ident_bf = consts.tile([128, 128], bf16)
            nc.vector.tensor_copy(ident_bf[:], ident_f[:])

            wencT = consts.tile([128, DC * EC * 128], bf16)
            wencT4 = wencT[:].rearrange("p (dj ei c) -> p dj ei c", dj=DC, ei=EC)
            enc_res0 = encres_pool.tile([128, ST * 1024], bf16, tag="encres")
            v_bf = consts.tile([128, DC], bf16)

            def load_enc0_pair(k):
                src0 = enc[0, k * 512:(k + 1) * 512, :]
                dst0 = enc_res0[:, k * 4096:(k + 1) * 4096]
                nc.sync.dma_start(
                    out=dst0.rearrange("p (t e) -> p t e", t=4),
                    in_=src0.rearrange("(t p) e -> p t e", p=128))

            def load_wenc_pair(k):
                nc.sync.dma_start(
                    out=wencT4[:, 2 * k:2 * k + 2, :, :],
                    in_=w_encTd[2 * k:2 * k + 2]
                        .rearrange("j (ei p) c -> p j ei c", p=128))

            load_enc0_pair(0)
            dhT = consts.tile([128, BL * DC], f32)
            wdec_stage = encres_pool.tile([128, DC * 1024], bf16, tag="encres")
            wdec3 = wdec_stage[:].rearrange("p (jc d) -> p jc d", jc=DC)
            decT = consts.tile([128, BL * DC], bf16)
            nc.sync.dma_start(out=decT[:], in_=decT_p[:, :])
            nc.sync.dma_start(
                out=wdec3[:, 0:DC // 2, :],
                in_=w_decT[0:D // 2].rearrange("(jc p) d -> p jc d", p=128))
            load_wenc_pair(0)
            nc.sync.dma_start(
                out=wdec3[:, DC // 2:DC, :],
                in_=w_decT[D // 2:D].rearrange("(jc p) d -> p jc d", p=128))
            nc.sync.dma_start(out=v_bf[:], in_=v_p[:, :])
            for k in range(1, 4):
                load_wenc_pair(k)
                load_enc0_pair(k)

            def emit_dh():
                ps_db = ps_ctx.tile([BL, 1024], f32, tag="ctx")
                for jc in range(DC):
                    for h in range(2):
                        nc.tensor.matmul(ps_db[:, h * 512:(h + 1) * 512],
                                         decT[:, jc * BL:(jc + 1) * BL],
                                         wdec3[:, jc, h * 512:(h + 1) * 512],
                                         start=(jc == 0), stop=(jc == DC - 1))
                dh_bd = small.tile([BL, 1024], f32, tag="dhbd", bufs=1)
                nc.scalar.copy(dh_bd[:], ps_db[:])
                ps_dt = ps_tp.tile([128, BL * DC], f32, tag="tp")
                for dj in range(DC):
                    nc.tensor.transpose(ps_dt[:, dj * BL:(dj + 1) * BL],
                                        dh_bd[:, dj * 128:(dj + 1) * 128],
                                        ident_f[:BL, :BL])
                nc.scalar.copy(dhT[:], ps_dt[:])

            def phase_E(b, c_callback=None, post_tp0=None, preloaded=None):
                if preloaded is not None:
                    enc_res = preloaded
                else:
                    enc_res = encres_pool.tile([128, ST * 1024], bf16,
                                               tag="encres")
                    for ld in range(4):
                        src = enc[b, ld * 512:(ld + 1) * 512, :]
                        dst = enc_res[:, ld * 4096:(ld + 1) * 4096]
                        nc.sync.dma_start(
                            out=dst.rearrange("p (t e) -> p t e", t=4),
                            in_=src.rearrange("(t p) e -> p t e", p=128))
                mask_ib = small.tile([1, S], i32, tag="mask")
                nc.sync.dma_start(out=mask_ib[:], in_=mask[b:b + 1, :])
                sm = rows.tile([1, S], f32, tag="row")
                wexp = rows.tile([1, S], f32, tag="row")
                se4 = small.tile([1, N_SC], f32, tag="pmax")
                for sc in range(N_SC):
                    if sc == 1 and c_callback is not None:
                        c_callback()
                    enct = []
                    for ep in range(EC // 2):
                        ps = ps_tp.tile([128, 1024], bf16, tag="tp")
                        for half in range(2):
                            ei = 2 * ep + half
                            for q in range(4):
                                t_abs = sc * 4 + q
                                src = enc_res[:, t_abs * 1024 + ei * 128:
                                              t_abs * 1024 + (ei + 1) * 128]
                                nc.tensor.transpose(
                                    ps[:, half * 512 + q * 128:
                                       half * 512 + (q + 1) * 128],
                                    src, ident_bf)
                        for half in range(2):
                            et = enct_pool.tile([128, 512], bf16, tag="enct")
                            nc.vector.tensor_copy(
                                et[:], ps[:, half * 512:(half + 1) * 512])
                            enct.append(et)
                    if sc == 0 and post_tp0 is not None:
                        post_tp0()
                    ens = []
                    for dj in range(DC):
                        psm = ps_mm.tile([128, 512], f32, tag="mm")
                        for ei in range(EC):
                            nc.tensor.matmul(
                                psm[:], wencT4[:, dj, ei, :], enct[ei][:],
                                start=(ei == 0), stop=(ei == EC - 1))
                        en = en_pool.tile([128, 512], bf16, tag="energy")
                        nc.scalar.activation(en[:], psm[:], Tanh,
                                             bias=dhT[:, dj * BL + b: dj * BL + b + 1])
                        ens.append(en)
                    ps_s = ps_vec.tile([1, 512], f32, tag="vec")
                    for dj in range(DC):
                        nc.tensor.matmul(ps_s[:], v_bf[:, dj:dj + 1],
                                         ens[dj][:],
                                         start=(dj == 0), stop=(dj == DC - 1))
                    pen = small.tile([1, 512], f32, tag="pen")
                    nc.vector.tensor_scalar(
                        out=pen[:], in0=mask_ib[:, sc * SC:(sc + 1) * SC],
                        scalar1=1e9, scalar2=-(1e9 + 8.0),
                        op0=_alu("mult"), op1=_alu("add"))
                    nc.vector.tensor_tensor(
                        out=sm[:, sc * SC:(sc + 1) * SC], in0=ps_s[:], in1=pen[:],
                        op=_alu("add"))
                    nc.scalar.activation(wexp[:, sc * SC:(sc + 1) * SC],
                                         sm[:, sc * SC:(sc + 1) * SC], Exp,
                                         accum_out=se4[:, sc:sc + 1])
                sumexp = small.tile([1, 1], f32, tag="sumexp")
                nc.vector.reduce_sum(sumexp[:], se4[:], axis=_axis("X"))
                rec = small.tile([1, 1], f32, tag="rec")
                nc.vector.reciprocal(rec[:], sumexp[:])
                wnorm = rows.tile([1, S], f32, tag="row")
                nc.vector.tensor_scalar_mul(wnorm[:], wexp[:], rec[:])
                nc.sync.dma_start(out=wts_out[b:b + 1, :],
                                  in_=wnorm[:].bitcast(f32r))
                return {"wexp": wexp, "rec": rec, "enc_res": enc_res}

            def phase_C(b, st_b):
                wexp, rec, enc_res = st_b["wexp"], st_b["rec"], st_b["enc_res"]
                ps_w = ps_tp.tile([128, ST], f32, tag="tp")
                for t in range(ST):
                    nc.tensor.transpose(ps_w[:, t:t + 1],
                                        wexp[:, t * 128:(t + 1) * 128],
                                        ident_f[:1, :1])
                wT = small.tile([128, ST], bf16, tag="wt")
                nc.vector.tensor_copy(wT[:], ps_w[:])

                psc = ps_ctx.tile([1, 1024], f32, tag="ctx")
                for st in range(ST):
                    rhs = enc_res[:, st * 1024:(st + 1) * 1024]
                    for h in range(2):
                        nc.tensor.matmul(psc[:, h * 512:(h + 1) * 512],
                                         wT[:, st:st + 1],
                                         rhs[:, h * 512:(h + 1) * 512],
                                         start=(st == 0), stop=(st == ST - 1))
                ctx_sb = small.tile([1, 1024], f32r, tag="ctxsb")
                nc.scalar.activation(ctx_sb[:, 0:512], psc[:, 0:512], Copy,
                                     scale=rec[:])
                nc.scalar.activation(ctx_sb[:, 512:1024], psc[:, 512:1024], Copy,
                                     scale=rec[:])
                nc.sync.dma_start(out=ctx_out[b:b + 1, :], in_=ctx_sb[:])

            states = {}
            for b in range(BL):
                cb = None
                if b >= 1:
                    cb = (lambda bb=b: phase_C(bb - 1, states[bb - 1]))
                states[b] = phase_E(b, c_callback=cb,
                                    post_tp0=emit_dh if b == 0 else None,
                                    preloaded=enc_res0 if b == 0 else None)
            phase_C(BL - 1, states[BL - 1])

    nc.compile()
    from concourse.bass_interp import get_hw_module
    nc.m = get_hw_module(nc.m)
    return nc


def get_nc():
    if "nc" not in _CACHE:
        _CACHE["nc"] = _build()
    return _CACHE["nc"]


def make_in_maps(dec_hidden, enc_outputs, mask, W_enc, W_dec, v):
    dec_hidden = np.ascontiguousarray(dec_hidden, dtype=np.float32)
    enc_outputs = np.ascontiguousarray(enc_outputs, dtype=np.float32)
    mask = np.ascontiguousarray(mask, dtype=np.int32)
    import ml_dtypes
    bf = ml_dtypes.bfloat16
    W_enc = np.asarray(W_enc, dtype=np.float32)
    W_dec = np.asarray(W_dec, dtype=np.float32)
    enc_bf = enc_outputs.astype(bf)
    w_encTd = np.ascontiguousarray(
        W_enc.T.reshape(E, DC, 128).transpose(1, 0, 2)).astype(bf)
    w_decT = np.ascontiguousarray(W_dec.T).astype(bf)
    v_p = np.ascontiguousarray(
        np.asarray(v, dtype=np.float32).reshape(DC, 128).T).astype(bf)

    in_maps = []
    for c in range(N_CORES):
        sl = slice(c * BL, (c + 1) * BL)
        decT_p = np.ascontiguousarray(
            dec_hidden[sl].T.reshape(DC, 128, BL).transpose(1, 0, 2)
            .reshape(128, DC * BL)).astype(bf)
        in_maps.append({
            "decT_p": decT_p,
            "enc": np.ascontiguousarray(enc_bf[sl]),
            "mask": np.ascontiguousarray(mask[sl]),
            "w_encTd": w_encTd, "w_decT": w_decT, "v_p": v_p,
        })
    return in_maps


def kernel(dec_hidden, enc_outputs, mask, W_enc, W_dec, v):
    from concourse import bass_utils

    nc = get_nc()
    in_maps = make_in_maps(dec_hidden, enc_outputs, mask, W_enc, W_dec, v)
    res = bass_utils.run_bass_kernel_spmd(nc, in_maps,
                                          core_ids=list(range(N_CORES)))
    context = np.concatenate([res.results[c]["context"] for c in range(N_CORES)])
    weights = np.concatenate([res.results[c]["weights"] for c in range(N_CORES)])
    return context.astype(np.float32), weights.astype(np.float32)
